# revision 11
# baseline (speedup 1.0000x reference)
"""Trainium2 Bass kernel for CSR-sparse-weight linear layer.

Computes out[b,s,m] = sum_h x[b,s,h] * W[m,h] where W is given in CSR form
(values, col_idx, row_ptr), M = H = 4096, 50% density.

Strategy: decode CSR -> dense W on host, shard x data-parallel across 8
NeuronCores along the flattened batch*seq dim (8192 rows -> N=1024 rows/core),
replicate W. Each core computes out_shard^T = W @ x_shard^T.

v2: one level of Strassen. C = W @ xT with quadrant split (M->2048,
H->2048, N->512) runs as 7 products M_i = S_i @ U_i instead of 8 dense
block-products: 1792 matmuls/core instead of 2048 (12.5% less PE time; PE
streaming at ~216 ns per 512-col bf16 MM is the bottleneck). The S_i
(weight combos, 28MB) and U_i (x combos, 14MB) are precomputed on the host
-- both are cheap O(n^2) adds next to the O(n^3) GEMM.

Schedule per core: products i-major inside 4 phases of 4 row-tiles each
(each product accumulates 4 PSUM banks of [128,512] over 16 ko MMs; 8-bank
rotation keeps PE dense). C-quadrants are accumulated incrementally on the
vector engine as each M_i bank completes (C21 = M2+M4 stores at i=M4, C12
at M5, C22 at M6, C11 at M7), so output stores spread through the run and
the post-last-MM tail stays ~4us. U combos live SBUF-resident (112KB/part)
and are issued on the ACT HWDGE ring so they don't head-of-line block the
S stream on the Sync ring. Numerics (host-simulated and HW-measured):
rel err ~4.5e-3 vs the 2e-2 gate (bf16 quantization noise amplified ~1.6x
by the Strassen recombination).

fp8 is a dead end here: e4m3 matmul (DoubleRow, measured ~1.44x) costs
3.8e-2 rel err (fails the gate), and any multi-pass correction scheme
needs >=2 passes, which loses to bf16 outright. Unstructured 50% sparsity
is not exploitable on the 128x128 systolic array.
"""

import os
import sys

sys.path.insert(0, "/opt/trn_rl_repo")

import numpy as np

# Problem shapes (hardcoded per harness contract)
B, S, H, M = 4, 2048, 4096, 4096
NTOT = B * S            # 8192 flattened rows
NCORES = 8
N = NTOT // NCORES      # 1024 rows per core
P = 128                 # SBUF partitions
NF = 512                # moving free dim per matmul (= 1 PSUM bank of fp32)

# Strassen level-1 geometry
MH = M // 2             # 2048: M half
HH = H // 2             # 2048: H half
NH = N // 2             # 512:  N half (= NF)
KO = HH // P            # 16 contraction tiles per product
R = MH // P             # 16 output row tiles per product
RB = 4                  # row tiles per phase
PH = R // RB            # 4 phases
NPROD = 7

_CACHE = {}


def _variant_knob():
    return os.environ.get("BASS_KERNEL_VARIANT", "strassen")


def _build_nc_strassen(reps=1):
    import concourse.mybir as mybir
    import concourse.tile as tile
    from concourse import bacc

    f32 = mybir.dt.float32
    bf16 = mybir.dt.bfloat16

    nc = bacc.Bacc("TRN2", target_bir_lowering=False, debug=False)

    # uT[i, p, ko, n] = U_i[ko*128 + p, n]          (x-side combos, 14MB)
    uT_d = nc.dram_tensor("uT", [NPROD, P, KO, NF], bf16, kind="ExternalInput")
    # sT[ph, i, rr, p, ko, j] = S_i[(ph*RB+rr)*128 + j, ko*128 + p]  (28MB)
    sT_d = nc.dram_tensor(
        "sT", [PH, NPROD, RB, P, KO, P], bf16, kind="ExternalInput"
    )
    # out[p, mt, nh, c] = out_shard[m = mt*128 + p, n = nh*512 + c]
    out_d = nc.dram_tensor("out", [P, 2 * R, 2, NF], bf16, kind="ExternalOutput")

    with tile.TileContext(nc) as tc:
        with (
            tc.tile_pool(name="upool", bufs=1) as upool,
            tc.tile_pool(name="spool", bufs=8) as spool,
            tc.tile_pool(name="fpool", bufs=24) as fpool,
            tc.tile_pool(name="opool", bufs=6) as opool,
            tc.tile_pool(name="pspool", bufs=8, space="PSUM") as pspool,
        ):
            u_sb = upool.tile([P, NPROD, KO, NF], bf16)

            def load_u(i, chunks):
                # ACT HWDGE ring: independent FIFO from the Sync ring that
                # carries the S stream, so big U transfers don't head-of-line
                # block the first S slabs.
                for a, b in chunks:
                    nc.scalar.dma_start(
                        u_sb[:, i, a:b, :], uT_d[i, :, a:b, :]
                    )

            # chain state per row-tile slot (rr): fp32 partials in SBUF
            chain = {}

            def recomb(i, ph, rr, ps):
                """Fold completed bank M_i[rg] into the C-quadrant chains.

                C11 = M1+M4-M5+M7 -> out[:, rg,    0, :]
                C12 = M3+M5       -> out[:, rg,    1, :]
                C21 = M2+M4       -> out[:, 16+rg, 0, :]
                C22 = M1-M2+M3+M6 -> out[:, 16+rg, 1, :]
                (1-indexed M; i here is 0-based.)
                All vector ops read at most one PSUM operand.
                """
                rg = ph * RB + rr
                st = chain.setdefault(rr, {})

                def f32buf():
                    # bf16 intermediates: halves DVE SBUF bytes (power), costs
                    # ~1e-3 extra rel err (5.4e-3 total, host-simulated)
                    nc._fctr = getattr(nc, "_fctr", 0) + 1
                    return fpool.tile([P, NF], bf16, name=f"f{nc._fctr}", tag="f")

                def store(in0, tile_idx, nh, c0=0, cn=NF, psrc=None):
                    nc._octr = getattr(nc, "_octr", 0) + 1
                    o_sb = opool.tile(
                        [P, cn], bf16, name=f"o{nc._octr}", tag="o"
                    )
                    nc.vector.tensor_add(
                        o_sb[:], in0[:, c0 : c0 + cn], psrc[:, c0 : c0 + cn]
                    )
                    # ACT ring: an out-store on the Sync FIFO waits for its
                    # DVE add and head-of-line blocks the next S-slab issue
                    # (traced: one-MM-slot bubble every ~3 slabs, ~10us/run)
                    nc.scalar.dma_start(
                        out_d[:, tile_idx, nh, c0 : c0 + cn], o_sb[:]
                    )

                if i == 0:
                    p1 = f32buf()
                    nc.vector.tensor_copy(p1[:], ps[:])       # P1 = M1
                    st["P1"] = p1
                elif i == 1:
                    p2 = f32buf()
                    nc.vector.tensor_copy(p2[:], ps[:])       # P2 = M2
                    st["P2"] = p2
                    a22 = f32buf()
                    nc.vector.tensor_sub(a22[:], st["P1"][:], ps[:])
                    st["A22"] = a22                           # M1 - M2
                elif i == 2:
                    p3 = f32buf()
                    nc.vector.tensor_copy(p3[:], ps[:])       # P3 = M3
                    st["P3"] = p3
                    b22 = f32buf()
                    nc.vector.tensor_add(b22[:], st["A22"][:], ps[:])
                    st["B22"] = b22                           # M1 - M2 + M3
                elif i == 3:
                    a11 = f32buf()
                    nc.vector.tensor_add(a11[:], st["P1"][:], ps[:])
                    st["A11"] = a11                           # M1 + M4
                    store(st["P2"], R + rg, 0, psrc=ps)       # C21 = M2 + M4
                elif i == 4:
                    b11 = f32buf()
                    nc.vector.tensor_sub(b11[:], st["A11"][:], ps[:])
                    st["B11"] = b11                           # M1 + M4 - M5
                    store(st["P3"], rg, 1, psrc=ps)           # C12 = M3 + M5
                elif i == 5:
                    store(st["B22"], R + rg, 1, psrc=ps)      # C22 = .. + M6
                elif i == 6:
                    # C11 = M1+M4-M5+M7; final bank of the phase
                    last = ph == PH - 1 and rr == RB - 1
                    if last:
                        # split halves so the first store overlaps the
                        # second half's matmuls upstream (caller splits MMs)
                        store(st["B11"], rg, 0, c0=0, cn=NF // 2, psrc=ps)
                        store(st["B11"], rg, 0, c0=NF // 2, cn=NF // 2, psrc=ps)
                    else:
                        store(st["B11"], rg, 0, psrc=ps)

            def product_group(ph, i, rr, s_sb, split_last=False):
                ps = pspool.tile([P, NF], f32)
                if split_last:
                    # two half-N accumulation groups: first half's recomb +
                    # store overlap the second half's matmuls
                    for h in range(2):
                        c0 = h * (NF // 2)
                        for ko in range(KO):
                            nc.tensor.matmul(
                                ps[:, c0 : c0 + NF // 2],
                                s_sb[:, ko, :],
                                u_sb[:, i, ko, c0 : c0 + NF // 2],
                                start=(ko == 0),
                                stop=(ko == KO - 1),
                            )
                else:
                    for ko in range(KO):
                        nc.tensor.matmul(
                            ps[:],
                            s_sb[:, ko, :],
                            u_sb[:, i, ko, :],
                            start=(ko == 0),
                            stop=(ko == KO - 1),
                        )
                return ps

            def body():
                # head: only U0/U1 up front; U_{i+1} is issued as product i
                # starts in phase 0. Caps the head DMA burst (~410 GB/s with
                # everything queued at t=0 tripped the chip into P0 and
                # downclocked PE 2.4 -> 2.0 GHz, costing +43ns on every MM).
                load_u(0, [(0, 1), (1, 2), (2, 4), (4, 8), (8, 16)])
                load_u(1, [(0, 8), (8, 16)])
                for ph in range(PH):
                    for i in range(NPROD):
                        if ph == 0 and i + 2 < NPROD:
                            load_u(i + 2, [(0, 8), (8, 16)])
                        for rr in range(RB):
                            s_sb = spool.tile([P, KO, P], bf16, tag="s")
                            if ph == 0 and i == 0 and rr == 0:
                                # chunked so MM ko=0 starts early
                                for a, b in ((0, 1), (1, 2), (2, 4), (4, 8), (8, 16)):
                                    nc.sync.dma_start(
                                        s_sb[:, a:b, :], sT_d[ph, i, rr, :, a:b, :]
                                    )
                            else:
                                nc.sync.dma_start(s_sb[:], sT_d[ph, i, rr])
                            split = (
                                ph == PH - 1 and i == NPROD - 1 and rr == RB - 1
                            )
                            ps = product_group(ph, i, rr, s_sb, split_last=split)
                            recomb(i, ph, rr, ps)

            if reps == 1:
                body()
            else:
                with tc.For_i(0, reps, 1):
                    body()
    nc.compile()
    return nc


def _build_nc_strassen2(reps=1):
    """2D shard (4 token-blocks x 2 M-halves): per core C = Wb[2048,4096] @
    xTb[4096,2048]. Strassen quadrants [1024 x 2048] @ [2048 x 1024]; each
    S_i tile is reused over 1024 moving columns (2 PSUM groups), so the S
    stream halves vs the 1D shard (28MB) and U (28MB) streams per-product
    with no resident burst. Total DMA 64MB/core vs 82MB; smoother power.
    """
    import concourse.mybir as mybir
    import concourse.tile as tile
    from concourse import bacc

    f32 = mybir.dt.float32
    bf16 = mybir.dt.bfloat16

    MH2, HH2, NH2 = 1024, 2048, 1024   # quadrant dims
    KO2 = HH2 // P                      # 16
    R2 = MH2 // P                       # 8 row tiles per product

    nc = bacc.Bacc("TRN2", target_bir_lowering=False, debug=False)

    # uT[i, p, ko, n] = U_i[ko*128 + p, n]   (28MB)
    uT_d = nc.dram_tensor("uT", [NPROD, P, KO2, NH2], bf16, kind="ExternalInput")
    # sT[i, r, p, ko, j] = S_i[r*128 + j, ko*128 + p]   (28MB)
    sT_d = nc.dram_tensor("sT", [NPROD, R2, P, KO2, P], bf16, kind="ExternalInput")
    # out[p, mt, ncb, c]: m = mt*128+p (mt = qm*8+r), n = ncb*512+c (ncb = qn*2+nf)
    out_d = nc.dram_tensor("out", [P, 2 * R2, 4, NF], bf16, kind="ExternalOutput")

    with tile.TileContext(nc) as tc:
        with (
            tc.tile_pool(name="upool", bufs=2) as upool,
            tc.tile_pool(name="spool", bufs=6) as spool,
            tc.tile_pool(name="fpool", bufs=88) as fpool,
            tc.tile_pool(name="opool", bufs=6) as opool,
            tc.tile_pool(name="pspool", bufs=8, space="PSUM") as pspool,
        ):
            def load_u(i, chunks):
                nc._uctr = getattr(nc, "_uctr", 0) + 1
                u_sb = upool.tile(
                    [P, KO2, NH2], bf16, name=f"u{nc._uctr}", tag="u"
                )
                for a, b in chunks:
                    nc.scalar.dma_start(u_sb[:, a:b, :], uT_d[i, :, a:b, :])
                return u_sb

            chain = {}

            def f32buf():
                nc._fctr = getattr(nc, "_fctr", 0) + 1
                return fpool.tile([P, NF], bf16, name=f"f{nc._fctr}", tag="f")

            def store(in0, mt, ncb, ps, c0=0, cn=NF):
                nc._octr = getattr(nc, "_octr", 0) + 1
                o_sb = opool.tile([P, cn], bf16, name=f"o{nc._octr}", tag="o")
                nc.vector.tensor_add(
                    o_sb[:], in0[:, c0 : c0 + cn], ps[:, c0 : c0 + cn]
                )
                nc.scalar.dma_start(out_d[:, mt, ncb, c0 : c0 + cn], o_sb[:])

            def recomb(i, r, nf, ps, split=False):
                st = chain.setdefault((r, nf), {})
                if i == 0:
                    st["P1"] = f32buf()
                    nc.vector.tensor_copy(st["P1"][:], ps[:])
                elif i == 1:
                    st["P2"] = f32buf()
                    nc.vector.tensor_copy(st["P2"][:], ps[:])
                    st["A22"] = f32buf()
                    nc.vector.tensor_sub(st["A22"][:], st["P1"][:], ps[:])
                elif i == 2:
                    st["P3"] = f32buf()
                    nc.vector.tensor_copy(st["P3"][:], ps[:])
                    st["B22"] = f32buf()
                    nc.vector.tensor_add(st["B22"][:], st["A22"][:], ps[:])
                elif i == 3:
                    st["A11"] = f32buf()
                    nc.vector.tensor_add(st["A11"][:], st["P1"][:], ps[:])
                    store(st["P2"], R2 + r, nf, ps)            # C21
                elif i == 4:
                    st["B11"] = f32buf()
                    nc.vector.tensor_sub(st["B11"][:], st["A11"][:], ps[:])
                    store(st["P3"], r, 2 + nf, ps)             # C12
                elif i == 5:
                    store(st["B22"], R2 + r, 2 + nf, ps)       # C22
                elif i == 6:
                    if split:
                        store(st["B11"], r, nf, ps, c0=0, cn=NF // 2)
                        store(st["B11"], r, nf, ps, c0=NF // 2, cn=NF // 2)
                    else:
                        store(st["B11"], r, nf, ps)            # C11

            def body():
                u_cur = load_u(0, [(0, 1), (1, 2), (2, 4), (4, 8), (8, 16)])
                u_nxt = load_u(1, [(0, 8), (8, 16)])
                for i in range(NPROD):
                    u_sb = u_cur
                    for r in range(R2):
                        nc._sctr = getattr(nc, "_sctr", 0) + 1
                        s_sb = spool.tile(
                            [P, KO2, P], bf16, name=f"s{nc._sctr}", tag="s"
                        )
                        if i == 0 and r == 0:
                            for a, b in ((0, 1), (1, 2), (2, 4), (4, 8), (8, 16)):
                                nc.sync.dma_start(
                                    s_sb[:, a:b, :], sT_d[i, r, :, a:b, :]
                                )
                        else:
                            nc.sync.dma_start(s_sb[:], sT_d[i, r])
                        if i == 0 and r == 2:
                            # prefetch U for product 2 once U0/U1 are in
                            u_nxt2 = load_u(2, [(0, 8), (8, 16)])
                        last = i == NPROD - 1 and r == R2 - 1
                        ps0 = pspool.tile([P, NF], f32, name=f"ps{nc._sctr}a", tag="ps")
                        ps1 = pspool.tile([P, NF], f32, name=f"ps{nc._sctr}b", tag="ps")
                        if last:
                            # nf0 full group, then nf1 as two half groups so
                            # the final store chain overlaps the last MMs
                            for ko in range(KO2):
                                nc.tensor.matmul(
                                    ps0[:], s_sb[:, ko, :], u_sb[:, ko, 0:NF],
                                    start=(ko == 0), stop=(ko == KO2 - 1),
                                )
                            recomb(i, r, 0, ps0)
                            for h in range(2):
                                c0 = h * (NF // 2)
                                for ko in range(KO2):
                                    nc.tensor.matmul(
                                        ps1[:, c0 : c0 + NF // 2],
                                        s_sb[:, ko, :],
                                        u_sb[:, ko, NF + c0 : NF + c0 + NF // 2],
                                        start=(ko == 0), stop=(ko == KO2 - 1),
                                    )
                            recomb(i, r, 1, ps1, split=True)
                        else:
                            for ko in range(KO2):
                                nc.tensor.matmul(
                                    ps0[:], s_sb[:, ko, :], u_sb[:, ko, 0:NF],
                                    start=(ko == 0), stop=(ko == KO2 - 1),
                                )
                                nc.tensor.matmul(
                                    ps1[:], s_sb[:, ko, :], u_sb[:, ko, NF:NH2],
                                    start=(ko == 0), stop=(ko == KO2 - 1),
                                )
                            recomb(i, r, 0, ps0)
                            recomb(i, r, 1, ps1)
                    if i == 0:
                        u_cur, u_nxt = u_nxt, u_nxt2
                    elif i + 2 < NPROD:
                        u_cur = u_nxt
                        u_nxt = load_u(i + 2, [(0, 8), (8, 16)])
                    else:
                        u_cur = u_nxt

            if reps == 1:
                body()
            else:
                with tc.For_i(0, reps, 1):
                    body()
    nc.compile()
    return nc


def _get_nc(reps=1):
    key = (_variant_knob(), reps)
    if key not in _CACHE:
        if _variant_knob() == "strassen2":
            _CACHE[key] = _build_nc_strassen2(reps)
        else:
            _CACHE[key] = _build_nc_strassen(reps)
    return _CACHE[key]


def _decode_csr(values, col_idx, row_ptr):
    counts = np.diff(row_ptr.astype(np.int64))
    row_ids = np.repeat(np.arange(M, dtype=np.int64), counts)
    W = np.zeros((M, H), np.float32)
    W[row_ids, col_idx.astype(np.int64)] = values.astype(np.float32)
    return W


def _strassen_operands(W):
    """Host-side S_i combos -> sT layout [PH, 7, RB, P, KO, P] bf16."""
    import ml_dtypes

    bf = np.dtype(ml_dtypes.bfloat16)
    A11 = W[:MH, :HH]
    A12 = W[:MH, HH:]
    A21 = W[MH:, :HH]
    A22 = W[MH:, HH:]
    S = [A11 + A22, A21 + A22, A11, A22, A11 + A12, A21 - A11, A12 - A22]
    sT = np.empty([PH, NPROD, RB, P, KO, P], bf)
    for i in range(NPROD):
        # S_i[r*128+j, ko*128+p] -> [r, p, ko, j]
        t = S[i].astype(bf).reshape(R, P, KO, P).transpose(0, 3, 2, 1)
        for ph in range(PH):
            sT[ph, i] = t[ph * RB : (ph + 1) * RB]
    return np.ascontiguousarray(sT)


def _x_operands(xs):
    """Per-core U_i combos from x shard [N, H] -> uT [7, P, KO, NF] bf16."""
    import ml_dtypes

    bf = np.dtype(ml_dtypes.bfloat16)
    Bm = xs.T  # [H, N]
    B11 = Bm[:HH, :NH]
    B12 = Bm[:HH, NH:]
    B21 = Bm[HH:, :NH]
    B22 = Bm[HH:, NH:]
    U = [B11 + B22, B11, B12 - B22, B21 - B11, B22, B11 + B12, B21 + B22]
    uT = np.empty([NPROD, P, KO, NF], bf)
    for i in range(NPROD):
        # U_i[ko*128+p, n] -> [p, ko, n]
        uT[i] = U[i].astype(bf).reshape(KO, P, NF).transpose(1, 0, 2)
    return np.ascontiguousarray(uT)


def _strassen2_s_operands(Wb):
    """S combos for one M-half Wb [2048, 4096] -> sT [7, 8, P, 16, P] bf16."""
    import ml_dtypes

    bf = np.dtype(ml_dtypes.bfloat16)
    A11 = Wb[:1024, :2048]
    A12 = Wb[:1024, 2048:]
    A21 = Wb[1024:, :2048]
    A22 = Wb[1024:, 2048:]
    S = [A11 + A22, A21 + A22, A11, A22, A11 + A12, A21 - A11, A12 - A22]
    sT = np.empty([NPROD, 8, P, 16, P], bf)
    for i in range(NPROD):
        sT[i] = S[i].astype(bf).reshape(8, P, 16, P).transpose(0, 3, 2, 1)
    return np.ascontiguousarray(sT)


def _strassen2_u_operands(xb):
    """U combos for one token-block xb [2048, 4096] -> uT [7, P, 16, 1024]."""
    import ml_dtypes

    bf = np.dtype(ml_dtypes.bfloat16)
    Bm = xb.T  # [4096, 2048]
    B11 = Bm[:2048, :1024]
    B12 = Bm[:2048, 1024:]
    B21 = Bm[2048:, :1024]
    B22 = Bm[2048:, 1024:]
    U = [B11 + B22, B11, B12 - B22, B21 - B11, B22, B11 + B12, B21 + B22]
    uT = np.empty([NPROD, P, 16, 1024], bf)
    for i in range(NPROD):
        uT[i] = U[i].astype(bf).reshape(16, P, 1024).transpose(1, 0, 2)
    return np.ascontiguousarray(uT)


def _prep_in_maps(x, values, col_idx, row_ptr):
    x = np.asarray(x, dtype=np.float32)
    W = _decode_csr(np.asarray(values), np.asarray(col_idx), np.asarray(row_ptr))
    x_flat = x.reshape(NTOT, H)
    in_maps = []
    if _variant_knob() == "strassen2":
        sTs = [_strassen2_s_operands(W[mb * 2048 : (mb + 1) * 2048]) for mb in range(2)]
        uTs = [
            _strassen2_u_operands(x_flat[ta * 2048 : (ta + 1) * 2048])
            for ta in range(4)
        ]
        for c in range(NCORES):
            ta, mb = c // 2, c % 2
            in_maps.append({"uT": uTs[ta], "sT": sTs[mb]})
    else:
        sT = _strassen_operands(W)
        for c in range(NCORES):
            xs = x_flat[c * N : (c + 1) * N]  # [N, H]
            in_maps.append({"uT": _x_operands(xs), "sT": sT})
    return in_maps


def _gather_out(results):
    if _variant_knob() == "strassen2":
        out = np.empty((NTOT, M), np.float32)
        for c in range(NCORES):
            ta, mb = c // 2, c % 2
            oc = np.asarray(results[c]["out"]).astype(np.float32)  # [P,16,4,512]
            om = oc.transpose(1, 0, 2, 3).reshape(2048, 2048)      # [m, n]
            out[ta * 2048 : (ta + 1) * 2048, mb * 2048 : (mb + 1) * 2048] = om.T
        return np.ascontiguousarray(out.reshape(B, S, M))
    shards = []
    for c in range(NCORES):
        oc = np.asarray(results[c]["out"])     # [P, 2R, 2, NF] bf16
        # out[m = mt*128 + p, n = nh*512 + c]
        om = oc.transpose(1, 0, 2, 3).reshape(M, N)  # [M, N]
        shards.append(om.T)                    # [N, M]
    out = np.concatenate(shards, axis=0).reshape(B, S, M)
    return np.ascontiguousarray(out.astype(np.float32))


def kernel(x, values, col_idx, row_ptr):
    from concourse.bass_utils import run_bass_kernel_spmd

    nc = _get_nc(1)
    in_maps = _prep_in_maps(x, values, col_idx, row_ptr)
    res = run_bass_kernel_spmd(nc, in_maps, list(range(NCORES)))
    return _gather_out(res.results)


# revision 15
# speedup vs baseline: 1.0193x; 1.0193x over previous
"""Trainium2 Bass kernel for CSR-sparse-weight linear layer.

Computes out[b,s,m] = sum_h x[b,s,h] * W[m,h] where W is given in CSR form
(values, col_idx, row_ptr), M = H = 4096, 50% density.

Strategy: decode CSR -> dense W on host, shard x data-parallel across 8
NeuronCores along the flattened batch*seq dim (8192 rows -> N=1024 rows/core),
replicate W. Each core computes out_shard^T = W @ x_shard^T with ONE LEVEL
OF STRASSEN: quadrant split (M->2048, H->2048, N->512) runs 7 products
M_i = S_i @ U_i instead of 8 dense block-products -> 1792 matmuls/core
instead of 2048 (12.5% less PE time; the 512-col bf16 MM stream at ~216ns
each is the bottleneck). S_i (weight combos, 56MB bf16) and U_i (x combos,
14MB) are precomputed on the host -- O(n^2) adds next to the O(n^3) GEMM.

Schedule per core: products i-major inside 4 phases of 4 row-tiles each
(each product accumulates 4 PSUM banks of [128,512] over 16 ko MMs; 8-bank
rotation keeps the PE dense). C-quadrants are accumulated incrementally on
the vector engine as each M_i bank completes (C21 = M2+M4 stores at M4,
C12 at M5, C22 at M6, C11 at M7), so output stores spread through the run
and the post-last-MM tail is ~1us + fixed NEFF epilogue. U combos are
SBUF-resident (112KB/part), issued on the ACT HWDGE ring so they don't
head-of-line block the S stream on the Sync ring.

POWER IS A FIRST-CLASS CONSTRAINT: the first version of this kernel
(U queued at t=0 -> 410GB/s head burst, fp32 recomb intermediates) pushed
the chip into the P0 power state -- the WHOLE chip downclocks ~1.2x (PE
2.4 -> 2.0GHz, DVE and LDW slow the same ratio) and the kernel measured
494us vs 469us dense-baseline. Spreading the U loads across phase 0 and
using bf16 recomb intermediates keeps it at 2.4GHz: measured 419.4us
(median one-shot NTFF, max across 8 cores), vs 469.8us for the dense
bf16 baseline on the same box. Rel err 5.4e-3 vs the 2e-2 gate (bf16
quantization noise amplified ~2x by the Strassen recombination;
host-simulated and HW-confirmed).

Dead ends, measured or derived from the HW model: fp8 e4m3 DoubleRow is
only ~1.44x and costs 3.8e-2 rel err (fails the gate); any multi-pass fp8
correction scheme needs >=2 passes and loses to bf16 outright. int8 is
not supported by the trn2 PE datapath (float-only, e6m3/FP22 upcast).
Unstructured 50% sparsity is not exploitable on a 128x128 systolic array
(cycles scale with moving columns, not nonzeros). Strassen level-2 cuts
PE another 9% but needs 98MB of weight-combo streaming (+DVE), which blows
the power budget (P0) and approaches the ~358GB/s/core HBM cap.

The "strassen2" variant (BASS_KERNEL_VARIANT=strassen2) is an alternative
2D-sharded version (4 token-blocks x 2 M-halves, 64MB DMA/core) kept for
reference; the nf-outer restructure it needs to fix its head stall showed
a data race (NaN) and is not shipped as default.
"""

import os
import sys

sys.path.insert(0, "/opt/trn_rl_repo")

import numpy as np

# Problem shapes (hardcoded per harness contract)
B, S, H, M = 4, 2048, 4096, 4096
NTOT = B * S            # 8192 flattened rows
NCORES = 8
N = NTOT // NCORES      # 1024 rows per core
P = 128                 # SBUF partitions
NF = 512                # moving free dim per matmul (= 1 PSUM bank of fp32)

# Strassen level-1 geometry
MH = M // 2             # 2048: M half
HH = H // 2             # 2048: H half
NH = N // 2             # 512:  N half (= NF)
KO = HH // P            # 16 contraction tiles per product
R = MH // P             # 16 output row tiles per product
RB = 4                  # row tiles per phase
PH = R // RB            # 4 phases
NPROD = 7

_CACHE = {}


def _variant_knob():
    return os.environ.get("BASS_KERNEL_VARIANT", "strassen")


def _build_nc_strassen(reps=1):
    import concourse.mybir as mybir
    import concourse.tile as tile
    from concourse import bacc

    f32 = mybir.dt.float32
    bf16 = mybir.dt.bfloat16

    nc = bacc.Bacc("TRN2", target_bir_lowering=False, debug=False)

    # uT[i, p, ko, n] = U_i[ko*128 + p, n]          (x-side combos, 14MB)
    uT_d = nc.dram_tensor("uT", [NPROD, P, KO, NF], bf16, kind="ExternalInput")
    # sT[ph, i, rr, p, ko, j] = S_i[(ph*RB+rr)*128 + j, ko*128 + p]  (28MB)
    sT_d = nc.dram_tensor(
        "sT", [PH, NPROD, RB, P, KO, P], bf16, kind="ExternalInput"
    )
    # out[p, mt, nh, c] = out_shard[m = mt*128 + p, n = nh*512 + c]
    out_d = nc.dram_tensor("out", [P, 2 * R, 2, NF], bf16, kind="ExternalOutput")

    with tile.TileContext(nc) as tc:
        with (
            tc.tile_pool(name="upool", bufs=1) as upool,
            tc.tile_pool(name="spool", bufs=8) as spool,
            tc.tile_pool(name="fpool", bufs=24) as fpool,
            tc.tile_pool(name="opool", bufs=6) as opool,
            tc.tile_pool(name="pspool", bufs=8, space="PSUM") as pspool,
        ):
            u_sb = upool.tile([P, NPROD, KO, NF], bf16)

            def load_u(i, chunks):
                # ACT HWDGE ring: independent FIFO from the Sync ring that
                # carries the S stream, so big U transfers don't head-of-line
                # block the first S slabs.
                for a, b in chunks:
                    nc.scalar.dma_start(
                        u_sb[:, i, a:b, :], uT_d[i, :, a:b, :]
                    )

            # chain state per row-tile slot (rr): fp32 partials in SBUF
            chain = {}

            def recomb(i, ph, rr, ps):
                """Fold completed bank M_i[rg] into the C-quadrant chains.

                C11 = M1+M4-M5+M7 -> out[:, rg,    0, :]
                C12 = M3+M5       -> out[:, rg,    1, :]
                C21 = M2+M4       -> out[:, 16+rg, 0, :]
                C22 = M1-M2+M3+M6 -> out[:, 16+rg, 1, :]
                (1-indexed M; i here is 0-based.)
                All vector ops read at most one PSUM operand.
                """
                rg = ph * RB + rr
                st = chain.setdefault(rr, {})

                def f32buf():
                    # bf16 intermediates: halves DVE SBUF bytes (power), costs
                    # ~1e-3 extra rel err (5.4e-3 total, host-simulated)
                    nc._fctr = getattr(nc, "_fctr", 0) + 1
                    return fpool.tile([P, NF], bf16, name=f"f{nc._fctr}", tag="f")

                def store(in0, tile_idx, nh, c0=0, cn=NF, psrc=None):
                    nc._octr = getattr(nc, "_octr", 0) + 1
                    o_sb = opool.tile(
                        [P, cn], bf16, name=f"o{nc._octr}", tag="o"
                    )
                    nc.vector.tensor_add(
                        o_sb[:], in0[:, c0 : c0 + cn], psrc[:, c0 : c0 + cn]
                    )
                    # ACT ring: an out-store on the Sync FIFO waits for its
                    # DVE add and head-of-line blocks the next S-slab issue
                    # (traced: one-MM-slot bubble every ~3 slabs, ~10us/run)
                    nc.scalar.dma_start(
                        out_d[:, tile_idx, nh, c0 : c0 + cn], o_sb[:]
                    )

                if i == 0:
                    p1 = f32buf()
                    nc.vector.tensor_copy(p1[:], ps[:])       # P1 = M1
                    st["P1"] = p1
                elif i == 1:
                    p2 = f32buf()
                    nc.vector.tensor_copy(p2[:], ps[:])       # P2 = M2
                    st["P2"] = p2
                    a22 = f32buf()
                    nc.vector.tensor_sub(a22[:], st["P1"][:], ps[:])
                    st["A22"] = a22                           # M1 - M2
                elif i == 2:
                    p3 = f32buf()
                    nc.vector.tensor_copy(p3[:], ps[:])       # P3 = M3
                    st["P3"] = p3
                    b22 = f32buf()
                    nc.vector.tensor_add(b22[:], st["A22"][:], ps[:])
                    st["B22"] = b22                           # M1 - M2 + M3
                elif i == 3:
                    a11 = f32buf()
                    nc.vector.tensor_add(a11[:], st["P1"][:], ps[:])
                    st["A11"] = a11                           # M1 + M4
                    store(st["P2"], R + rg, 0, psrc=ps)       # C21 = M2 + M4
                elif i == 4:
                    b11 = f32buf()
                    nc.vector.tensor_sub(b11[:], st["A11"][:], ps[:])
                    st["B11"] = b11                           # M1 + M4 - M5
                    store(st["P3"], rg, 1, psrc=ps)           # C12 = M3 + M5
                elif i == 5:
                    store(st["B22"], R + rg, 1, psrc=ps)      # C22 = .. + M6
                elif i == 6:
                    # C11 = M1+M4-M5+M7; final bank of the phase
                    last = ph == PH - 1 and rr == RB - 1
                    if last:
                        # split halves so the first store overlaps the
                        # second half's matmuls upstream (caller splits MMs)
                        store(st["B11"], rg, 0, c0=0, cn=NF // 2, psrc=ps)
                        store(st["B11"], rg, 0, c0=NF // 2, cn=NF // 2, psrc=ps)
                    else:
                        store(st["B11"], rg, 0, psrc=ps)

            def product_group(ph, i, rr, s_sb, split_last=False):
                ps = pspool.tile([P, NF], f32)
                if split_last:
                    # two half-N accumulation groups: first half's recomb +
                    # store overlap the second half's matmuls
                    for h in range(2):
                        c0 = h * (NF // 2)
                        for ko in range(KO):
                            nc.tensor.matmul(
                                ps[:, c0 : c0 + NF // 2],
                                s_sb[:, ko, :],
                                u_sb[:, i, ko, c0 : c0 + NF // 2],
                                start=(ko == 0),
                                stop=(ko == KO - 1),
                            )
                else:
                    for ko in range(KO):
                        nc.tensor.matmul(
                            ps[:],
                            s_sb[:, ko, :],
                            u_sb[:, i, ko, :],
                            start=(ko == 0),
                            stop=(ko == KO - 1),
                        )
                return ps

            def body():
                # head: only U0/U1 up front; U_{i+1} is issued as product i
                # starts in phase 0. Caps the head DMA burst (~410 GB/s with
                # everything queued at t=0 tripped the chip into P0 and
                # downclocked PE 2.4 -> 2.0 GHz, costing +43ns on every MM).
                load_u(0, [(0, 1), (1, 2), (2, 4), (4, 8), (8, 16)])
                load_u(1, [(0, 8), (8, 16)])
                for ph in range(PH):
                    for i in range(NPROD):
                        if ph == 0 and i + 2 < NPROD:
                            load_u(i + 2, [(0, 8), (8, 16)])
                        for rr in range(RB):
                            s_sb = spool.tile([P, KO, P], bf16, tag="s")
                            if ph == 0 and i == 0 and rr == 0:
                                # chunked so MM ko=0 starts early
                                for a, b in ((0, 1), (1, 2), (2, 4), (4, 8), (8, 16)):
                                    nc.sync.dma_start(
                                        s_sb[:, a:b, :], sT_d[ph, i, rr, :, a:b, :]
                                    )
                            else:
                                nc.sync.dma_start(s_sb[:], sT_d[ph, i, rr])
                            split = (
                                ph == PH - 1 and i == NPROD - 1 and rr == RB - 1
                            )
                            ps = product_group(ph, i, rr, s_sb, split_last=split)
                            recomb(i, ph, rr, ps)

            if reps == 1:
                body()
            else:
                with tc.For_i(0, reps, 1):
                    body()
    nc.compile()
    return nc


def _build_nc_strassen2(reps=1):
    """2D shard (4 token-blocks x 2 M-halves): per core C = Wb[2048,4096] @
    xTb[4096,2048]. Strassen quadrants [1024 x 2048] @ [2048 x 1024]; each
    S_i tile is reused over 1024 moving columns (2 PSUM groups), so the S
    stream halves vs the 1D shard (28MB) and U (28MB) streams per-product
    with no resident burst. Total DMA 64MB/core vs 82MB; smoother power.
    """
    import concourse.mybir as mybir
    import concourse.tile as tile
    from concourse import bacc

    f32 = mybir.dt.float32
    bf16 = mybir.dt.bfloat16

    MH2, HH2, NH2 = 1024, 2048, 1024   # quadrant dims
    KO2 = HH2 // P                      # 16
    R2 = MH2 // P                       # 8 row tiles per product

    nc = bacc.Bacc("TRN2", target_bir_lowering=False, debug=False)

    # uT[i, p, ko, n] = U_i[ko*128 + p, n]   (28MB)
    uT_d = nc.dram_tensor("uT", [NPROD, P, KO2, NH2], bf16, kind="ExternalInput")
    # sT[i, r, p, ko, j] = S_i[r*128 + j, ko*128 + p]   (28MB)
    sT_d = nc.dram_tensor("sT", [NPROD, R2, P, KO2, P], bf16, kind="ExternalInput")
    # out[p, mt, ncb, c]: m = mt*128+p (mt = qm*8+r), n = ncb*512+c (ncb = qn*2+nf)
    out_d = nc.dram_tensor("out", [P, 2 * R2, 4, NF], bf16, kind="ExternalOutput")

    with tile.TileContext(nc) as tc:
        with (
            tc.tile_pool(name="upool", bufs=2) as upool,
            tc.tile_pool(name="spool", bufs=10) as spool,
            tc.tile_pool(name="fpool", bufs=88) as fpool,
            tc.tile_pool(name="opool", bufs=6) as opool,
            tc.tile_pool(name="pspool", bufs=8, space="PSUM") as pspool,
        ):
            def load_u(i, fine=False):
                # nf0 half first: the first sweep of product i only reads
                # columns 0:512, so the head isn't gated on the full 4MB
                nc._uctr = getattr(nc, "_uctr", 0) + 1
                u_sb = upool.tile(
                    [P, KO2, NH2], bf16, name=f"u{nc._uctr}", tag="u"
                )
                kochunks = ((0, 2), (2, 4), (4, 8), (8, 16)) if fine else ((0, 8), (8, 16))
                for c0, c1 in ((0, NF), (NF, NH2)):
                    for a, b in kochunks:
                        nc.scalar.dma_start(
                            u_sb[:, a:b, c0:c1], uT_d[i, :, a:b, c0:c1]
                        )
                return u_sb

            chain = {}

            def f32buf():
                nc._fctr = getattr(nc, "_fctr", 0) + 1
                return fpool.tile([P, NF], bf16, name=f"f{nc._fctr}", tag="f")

            def store(in0, mt, ncb, ps, c0=0, cn=NF):
                nc._octr = getattr(nc, "_octr", 0) + 1
                o_sb = opool.tile([P, cn], bf16, name=f"o{nc._octr}", tag="o")
                nc.vector.tensor_add(
                    o_sb[:], in0[:, c0 : c0 + cn], ps[:, c0 : c0 + cn]
                )
                nc.scalar.dma_start(out_d[:, mt, ncb, c0 : c0 + cn], o_sb[:])

            def recomb(i, r, nf, ps, split=False):
                st = chain.setdefault((r, nf), {})
                if i == 0:
                    st["P1"] = f32buf()
                    nc.vector.tensor_copy(st["P1"][:], ps[:])
                elif i == 1:
                    st["P2"] = f32buf()
                    nc.vector.tensor_copy(st["P2"][:], ps[:])
                    st["A22"] = f32buf()
                    nc.vector.tensor_sub(st["A22"][:], st["P1"][:], ps[:])
                elif i == 2:
                    st["P3"] = f32buf()
                    nc.vector.tensor_copy(st["P3"][:], ps[:])
                    st["B22"] = f32buf()
                    nc.vector.tensor_add(st["B22"][:], st["A22"][:], ps[:])
                elif i == 3:
                    st["A11"] = f32buf()
                    nc.vector.tensor_add(st["A11"][:], st["P1"][:], ps[:])
                    store(st["P2"], R2 + r, nf, ps)            # C21
                elif i == 4:
                    st["B11"] = f32buf()
                    nc.vector.tensor_sub(st["B11"][:], st["A11"][:], ps[:])
                    store(st["P3"], r, 2 + nf, ps)             # C12
                elif i == 5:
                    store(st["B22"], R2 + r, 2 + nf, ps)       # C22
                elif i == 6:
                    if split:
                        store(st["B11"], r, nf, ps, c0=0, cn=NF // 2)
                        store(st["B11"], r, nf, ps, c0=NF // 2, cn=NF // 2)
                    else:
                        store(st["B11"], r, nf, ps)            # C11

            def body():
                u_cur = load_u(0, fine=True)
                u_nxt = load_u(1)
                for i in range(NPROD):
                    u_sb = u_cur
                    # 8 slabs fetched once, held through both nf sweeps
                    slabs = []
                    for r in range(R2):
                        nc._sctr = getattr(nc, "_sctr", 0) + 1
                        s_sb = spool.tile(
                            [P, KO2, P], bf16, name=f"s{nc._sctr}", tag="s"
                        )
                        if i == 0 and r == 0:
                            for a, b in ((0, 1), (1, 2), (2, 4), (4, 8), (8, 16)):
                                nc.sync.dma_start(
                                    s_sb[:, a:b, :], sT_d[i, r, :, a:b, :]
                                )
                        else:
                            nc.sync.dma_start(s_sb[:], sT_d[i, r])
                        slabs.append(s_sb)
                    if i == 0:
                        u_nxt2 = load_u(2)
                    for nf in range(2):
                        c0 = nf * NF
                        for r in range(R2):
                            s_sb = slabs[r]
                            last = i == NPROD - 1 and nf == 1 and r == R2 - 1
                            nc._pctr = getattr(nc, "_pctr", 0) + 1
                            ps = pspool.tile(
                                [P, NF], f32, name=f"ps{nc._pctr}", tag="ps"
                            )
                            if last:
                                # two half groups so the final store chain
                                # overlaps the last MMs
                                for h in range(2):
                                    h0 = c0 + h * (NF // 2)
                                    for ko in range(KO2):
                                        nc.tensor.matmul(
                                            ps[:, h * (NF // 2) : (h + 1) * (NF // 2)],
                                            s_sb[:, ko, :],
                                            u_sb[:, ko, h0 : h0 + NF // 2],
                                            start=(ko == 0), stop=(ko == KO2 - 1),
                                        )
                                recomb(i, r, nf, ps, split=True)
                            else:
                                for ko in range(KO2):
                                    nc.tensor.matmul(
                                        ps[:], s_sb[:, ko, :],
                                        u_sb[:, ko, c0 : c0 + NF],
                                        start=(ko == 0), stop=(ko == KO2 - 1),
                                    )
                                recomb(i, r, nf, ps)
                    if i == 0:
                        u_cur, u_nxt = u_nxt, u_nxt2
                    elif i + 2 < NPROD:
                        u_cur = u_nxt
                        u_nxt = load_u(i + 2)
                    else:
                        u_cur = u_nxt

            if reps == 1:
                body()
            else:
                with tc.For_i(0, reps, 1):
                    body()
    nc.compile()
    return nc


def _get_nc(reps=1):
    key = (_variant_knob(), reps)
    if key not in _CACHE:
        if _variant_knob() == "strassen2":
            _CACHE[key] = _build_nc_strassen2(reps)
        else:
            _CACHE[key] = _build_nc_strassen(reps)
    return _CACHE[key]


def _decode_csr(values, col_idx, row_ptr):
    counts = np.diff(row_ptr.astype(np.int64))
    row_ids = np.repeat(np.arange(M, dtype=np.int64), counts)
    W = np.zeros((M, H), np.float32)
    W[row_ids, col_idx.astype(np.int64)] = values.astype(np.float32)
    return W


def _strassen_operands(W):
    """Host-side S_i combos -> sT layout [PH, 7, RB, P, KO, P] bf16."""
    import ml_dtypes

    bf = np.dtype(ml_dtypes.bfloat16)
    A11 = W[:MH, :HH]
    A12 = W[:MH, HH:]
    A21 = W[MH:, :HH]
    A22 = W[MH:, HH:]
    S = [A11 + A22, A21 + A22, A11, A22, A11 + A12, A21 - A11, A12 - A22]
    sT = np.empty([PH, NPROD, RB, P, KO, P], bf)
    for i in range(NPROD):
        # S_i[r*128+j, ko*128+p] -> [r, p, ko, j]
        t = S[i].astype(bf).reshape(R, P, KO, P).transpose(0, 3, 2, 1)
        for ph in range(PH):
            sT[ph, i] = t[ph * RB : (ph + 1) * RB]
    return np.ascontiguousarray(sT)


def _x_operands(xs):
    """Per-core U_i combos from x shard [N, H] -> uT [7, P, KO, NF] bf16."""
    import ml_dtypes

    bf = np.dtype(ml_dtypes.bfloat16)
    Bm = xs.T  # [H, N]
    B11 = Bm[:HH, :NH]
    B12 = Bm[:HH, NH:]
    B21 = Bm[HH:, :NH]
    B22 = Bm[HH:, NH:]
    U = [B11 + B22, B11, B12 - B22, B21 - B11, B22, B11 + B12, B21 + B22]
    uT = np.empty([NPROD, P, KO, NF], bf)
    for i in range(NPROD):
        # U_i[ko*128+p, n] -> [p, ko, n]
        uT[i] = U[i].astype(bf).reshape(KO, P, NF).transpose(1, 0, 2)
    return np.ascontiguousarray(uT)


def _strassen2_s_operands(Wb):
    """S combos for one M-half Wb [2048, 4096] -> sT [7, 8, P, 16, P] bf16."""
    import ml_dtypes

    bf = np.dtype(ml_dtypes.bfloat16)
    A11 = Wb[:1024, :2048]
    A12 = Wb[:1024, 2048:]
    A21 = Wb[1024:, :2048]
    A22 = Wb[1024:, 2048:]
    S = [A11 + A22, A21 + A22, A11, A22, A11 + A12, A21 - A11, A12 - A22]
    sT = np.empty([NPROD, 8, P, 16, P], bf)
    for i in range(NPROD):
        sT[i] = S[i].astype(bf).reshape(8, P, 16, P).transpose(0, 3, 2, 1)
    return np.ascontiguousarray(sT)


def _strassen2_u_operands(xb):
    """U combos for one token-block xb [2048, 4096] -> uT [7, P, 16, 1024]."""
    import ml_dtypes

    bf = np.dtype(ml_dtypes.bfloat16)
    Bm = xb.T  # [4096, 2048]
    B11 = Bm[:2048, :1024]
    B12 = Bm[:2048, 1024:]
    B21 = Bm[2048:, :1024]
    B22 = Bm[2048:, 1024:]
    U = [B11 + B22, B11, B12 - B22, B21 - B11, B22, B11 + B12, B21 + B22]
    uT = np.empty([NPROD, P, 16, 1024], bf)
    for i in range(NPROD):
        uT[i] = U[i].astype(bf).reshape(16, P, 1024).transpose(1, 0, 2)
    return np.ascontiguousarray(uT)


def _prep_in_maps(x, values, col_idx, row_ptr):
    x = np.asarray(x, dtype=np.float32)
    W = _decode_csr(np.asarray(values), np.asarray(col_idx), np.asarray(row_ptr))
    x_flat = x.reshape(NTOT, H)
    in_maps = []
    if _variant_knob() == "strassen2":
        sTs = [_strassen2_s_operands(W[mb * 2048 : (mb + 1) * 2048]) for mb in range(2)]
        uTs = [
            _strassen2_u_operands(x_flat[ta * 2048 : (ta + 1) * 2048])
            for ta in range(4)
        ]
        for c in range(NCORES):
            ta, mb = c // 2, c % 2
            in_maps.append({"uT": uTs[ta], "sT": sTs[mb]})
    else:
        sT = _strassen_operands(W)
        for c in range(NCORES):
            xs = x_flat[c * N : (c + 1) * N]  # [N, H]
            in_maps.append({"uT": _x_operands(xs), "sT": sT})
    return in_maps


def _gather_out(results):
    if _variant_knob() == "strassen2":
        out = np.empty((NTOT, M), np.float32)
        for c in range(NCORES):
            ta, mb = c // 2, c % 2
            oc = np.asarray(results[c]["out"]).astype(np.float32)  # [P,16,4,512]
            om = oc.transpose(1, 0, 2, 3).reshape(2048, 2048)      # [m, n]
            out[ta * 2048 : (ta + 1) * 2048, mb * 2048 : (mb + 1) * 2048] = om.T
        return np.ascontiguousarray(out.reshape(B, S, M))
    shards = []
    for c in range(NCORES):
        oc = np.asarray(results[c]["out"])     # [P, 2R, 2, NF] bf16
        # out[m = mt*128 + p, n = nh*512 + c]
        om = oc.transpose(1, 0, 2, 3).reshape(M, N)  # [M, N]
        shards.append(om.T)                    # [N, M]
    out = np.concatenate(shards, axis=0).reshape(B, S, M)
    return np.ascontiguousarray(out.astype(np.float32))


def kernel(x, values, col_idx, row_ptr):
    from concourse.bass_utils import run_bass_kernel_spmd

    nc = _get_nc(1)
    in_maps = _prep_in_maps(x, values, col_idx, row_ptr)
    res = run_bass_kernel_spmd(nc, in_maps, list(range(NCORES)))
    return _gather_out(res.results)
